# revision 2
# baseline (speedup 1.0000x reference)
"""Causal single-head attention (B=4, S=4096, D=512, dk=64) on 8 Trainium2
NeuronCores via Bass/Tile.

v6 = v1's balanced bf16 attention pipeline (scores/exp/masks/PV identical)
with three orthogonal upgrades:
  - projections run as fp8e4 DoubleRow matmuls (x/W fp8, weights x16),
    halving the projection instruction count.  x1 chunk 0 and x2 chunks
    0/1 stay bf16 (few-key softmax rows can't average away fp8 noise).
  - input DMA: fp8 bulk (half the bytes), consumption-ordered triggers,
    chunk-pair granularity for the fp8 x2 stream.
  - output: one DMA per 512-row job instead of four; oT psum->sbuf copy
    moved to the ACT engine (its slack engine in this balance).

Scaling: weights x16 => q,k,v carry x16, scores x256 (exp scale 0.125/256),
v ones-column = 16 so numerator and denominator share the x16 factor.
"""
import os
import numpy as np
import ml_dtypes

import bass_rust
import concourse.bass as bass
import concourse.tile as tile
from concourse import mybir
from concourse.bass_utils import run_bass_kernel_spmd
from concourse.masks import make_identity

# ---------------------------------------------------------------- constants
P = 128
D = 512
DK = 64
S = 4096
B = 4
CH = 512
NJ = 4
KD = D // P
NSK = S // P
SQ = NJ * CH
N_CORES = 8
VP = 80
WM = 5 * DK      # packed weight columns [Wq|Wq|Wk|Wk|Wv]
WSCALE = 16.0
ESCALE = 0.125 / (WSCALE * WSCALE)

F32 = mybir.dt.float32
BF16 = mybir.dt.bfloat16
F8 = mybir.dt.float8e4
DRM = mybir.MatmulPerfMode.DoubleRow
NP8 = ml_dtypes.float8_e4m3
NPB = ml_dtypes.bfloat16

_CFG = {
    "trace": os.environ.get("K_TRACE", "0") == "1",
}


# ------------------------------------------------- walrus codegen workarounds
def _patch_tile_drain():
    """This neuronxcc rejects >1 sync wait on a CTRL (Drain) instruction;
    TileContext's tail drain carries one wait per live semaphore.  Split the
    waits onto dedicated SP nops, one wait each."""
    from concourse.tile import TileContext

    if getattr(TileContext, "_drain_patched", False):
        return

    def _patched(self, tick_clock, wait_clock):
        nc = self.nc
        probe = nc.sync.nop(nofuse=True, hint="tail_wait_probe")
        wait_clock.add_sem_waits(
            probe.ins, bass_rust.ScopedClock({None: tick_clock.global_clock})
        )
        si = probe.ins.sync_info
        waits = list(si.on_wait) if si is not None else []
        probe.ins.sync_info = bass_rust.SyncInfo(on_wait=waits[:1], on_update=[])
        for w in waits[1:]:
            carrier = nc.sync.nop(nofuse=True, hint="tail_wait")
            carrier.ins.sync_info = bass_rust.SyncInfo(on_wait=[w], on_update=[])
        nc.sync.drain()

        nc.all_engine_barrier()
        assert self.sems is not None
        popped = nc._tile_sem_poison_stack.pop()
        assert popped is self._sem_poison
        nc.clear_and_free_semaphores(list(self.sems.allocated().values()))
        nc.all_engine_barrier()

    TileContext._drain_and_barrier = _patched
    TileContext._drain_patched = True


def _split_sync_waits(nc, max_waits: int = 1):
    """walrus here rejects >1 sync wait on at least CTRL and S3_LW (weight
    load) instruction structs.  Hoist excess waits onto same-engine NOPs
    placed immediately before the instruction (engine streams execute block
    order, so the waits still gate the instruction)."""
    counter = [0]
    for fn in nc.m.functions:
        for bb in fn.blocks:
            changed = False
            new = []
            for inst in bb.instructions:
                si = inst.sync_info
                waits = list(si.on_wait) if si is not None else []
                if len(waits) > max_waits:
                    changed = True
                    for w in waits[:-max_waits]:
                        counter[0] += 1
                        nop = bass_rust.InstNoOp(
                            name=f"I-waitsplit-{counter[0]}", engine=inst.engine
                        )
                        nop.bass_nofuse = True
                        nop.sync_info = bass_rust.SyncInfo(
                            on_wait=[w], on_update=[]
                        )
                        new.append(nop)
                    inst.sync_info = bass_rust.SyncInfo(
                        on_wait=waits[-max_waits:], on_update=list(si.on_update)
                    )
                new.append(inst)
            if changed:
                bb.instructions = new


# ---------------------------------------------------------------- program
def _build_program(causal: bool):
    _patch_tile_drain()
    nc = bass.Bass()

    # bf16 side-channel: x1 chunk 0, x2 chunks 0-1 (first 1024 rows/keys)
    x1b = nc.declare_dram_parameter("x1b", [P, KD * CH], BF16, isOutput=False)
    x2b = nc.declare_dram_parameter("x2b", [2, P, KD * CH], BF16,
                                    isOutput=False)
    x1e = nc.declare_dram_parameter("x1e", [SQ // CH - 1, P, KD * CH], F8,
                                    isOutput=False)
    x2e = nc.declare_dram_parameter("x2e", [S // CH - 2, P, KD * CH], F8,
                                    isOutput=False)
    wallb = nc.declare_dram_parameter("wallb", [P, KD * WM], BF16,
                                      isOutput=False)
    wall8 = nc.declare_dram_parameter("wall8", [P, KD * WM], F8,
                                      isOutput=False)
    ball = nc.declare_dram_parameter("ball", [P, 3], F32, isOutput=False)
    masks = nc.declare_dram_parameter("masks", [8, P, CH], BF16,
                                      isOutput=False)
    ones = nc.declare_dram_parameter("ones", [P, NSK], BF16, isOutput=False)
    out = nc.declare_dram_parameter("out", [NJ, CH, DK], F32, isOutput=True)

    E = [8 * j + 8 for j in range(NJ)] if causal else [NSK] * NJ

    Exp = mybir.ActivationFunctionType.Exp

    def bias_relu(dst, src_psum, bias_sb):
        nc.vector.tensor_scalar(
            dst, src_psum, bias_sb, 0.0,
            mybir.AluOpType.add, mybir.AluOpType.max,
        )

    with tile.TileContext(nc) as tc:
        with (
            tc.tile_pool(name="const", bufs=1) as const,
            tc.tile_pool(name="xin", bufs=1) as xin,
            tc.tile_pool(name="resident", bufs=1) as res,
            tc.tile_pool(name="attn", bufs=6) as attn,
            tc.tile_pool(name="ostage", bufs=4) as ostage,
            tc.tile_pool(name="outps", bufs=2, space="PSUM") as outps,
            tc.tile_pool(name="pps", bufs=2, space="PSUM") as pps,
            tc.tile_pool(name="sps", bufs=2, space="PSUM") as sps,
        ):
            # ---------------- constants / residents
            wb_sb = const.tile([P, KD, WM], BF16)
            w8_sb = const.tile([P, KD, WM], F8)
            b_sb = const.tile([P, 3], F32)
            ident = const.tile([P, P], F32)
            make_identity(nc, ident)
            identv = const.tile([P, P], BF16)
            make_identity(nc, identv)

            qT_sb = res.tile([P, SQ], BF16)
            kT_sb = res.tile([P, S], BF16)
            vT_sb = res.tile([DK, S], BF16)
            v_sb = res.tile([P, NSK, VP], BF16)

            # ---- input DMAs in consumption order; first chunks split
            # per-kd so the first projection matmuls start early.
            x1fp = [
                xin.tile([P, 2, CH], BF16, name=f"x1fp{g}", tag=f"x1fp{g}")
                for g in range(2)
            ]
            x2fp = [
                xin.tile([P, 2, CH], BF16, name=f"x2fp{g}", tag=f"x2fp{g}")
                for g in range(2)
            ]
            x1_first = [x1fp[k // 2][:, k % 2, :] for k in range(KD)]
            x2_first = [x2fp[k // 2][:, k % 2, :] for k in range(KD)]
            x2b1 = xin.tile([P, KD, CH], BF16, name="x2b1", tag="x2b1")
            x1t8 = xin.tile([P, SQ // CH - 1, KD, CH], F8, name="x1t8",
                            tag="x1t8")
            x2t8s = [
                xin.tile([P, 2, KD, CH], F8, name=f"x2t8{g}", tag=f"x2t8{g}")
                for g in range(3)
            ]

            wbv = wallb.rearrange("p (kd m) -> p kd m", kd=KD)
            nc.sync.dma_start(out=wb_sb[:, :, 0:P], in_=wbv[:, :, 0:P])
            x1v0 = x1b.rearrange("p (kd s) -> p kd s", kd=KD)
            x2v0 = x2b[0].rearrange("p (kd s) -> p kd s", kd=KD)
            for g in range(2):
                nc.sync.dma_start(
                    out=x1fp[g], in_=x1v0[:, 2 * g:2 * g + 2, :]
                )
            nc.sync.dma_start(
                out=wb_sb[:, :, P:WM], in_=wbv[:, :, P:WM]
            )
            for g in range(2):
                nc.sync.dma_start(
                    out=x2fp[g], in_=x2v0[:, 2 * g:2 * g + 2, :]
                )
            nc.sync.dma_start(out=b_sb, in_=ball[:, :])
            nc.sync.dma_start(
                out=x2b1, in_=x2b[1].rearrange("p (kd s) -> p kd s", kd=KD)
            )
            nc.sync.dma_start(
                out=v_sb[:, :, DK:DK + 1],
                in_=ones.rearrange("p (n o) -> p n o", o=1),
            )
            if causal:
                masks_sb = const.tile([P, 8, CH], BF16)
                nc.sync.dma_start(
                    out=masks_sb, in_=masks.rearrange("m p s -> p m s")
                )
            nc.sync.dma_start(
                out=w8_sb, in_=wall8.rearrange("p (kd m) -> p kd m", kd=KD)
            )
            nc.sync.dma_start(
                out=x1t8,
                in_=x1e.rearrange("c p (kd s) -> p c kd s", kd=KD),
            )
            x2ev = x2e.rearrange("(g c) p (kd s) -> g p c kd s", c=2, kd=KD)
            for g in range(3):
                nc.sync.dma_start(out=x2t8s[g], in_=x2ev[g])

            def proj_bf16(pq, wcol0, wcolw, xs):
                for kd in range(KD):
                    nc.tensor.matmul(
                        pq, wb_sb[:, kd, wcol0:wcol0 + wcolw], xs(kd),
                        start=(kd == 0), stop=(kd == KD - 1),
                    )

            def proj_dr(pq, wcol0, wcolw, x3):
                for i in range(KD // 2):
                    nc.tensor.matmul(
                        pq, w8_sb[:, 2 * i:2 * i + 2, wcol0:wcol0 + wcolw],
                        x3[:, 2 * i:2 * i + 2, :],
                        start=(i == 0), stop=(i == KD // 2 - 1),
                        perf_mode=DRM,
                    )

            def proj_q_chunk(ch):
                pq = pps.tile([P, CH], F32, tag="pps")
                if ch == 0:
                    proj_bf16(pq, 0, P, lambda kd: x1_first[kd])
                else:
                    proj_dr(pq, 0, P, x1t8[:, ch - 1])
                bias_relu(qT_sb[:, ch * CH:(ch + 1) * CH], pq, b_sb[:, 0:1])

            def proj_kv_chunk(ch):
                pk = pps.tile([P, CH], F32, tag="pps")
                if ch == 0:
                    proj_bf16(pk, 2 * DK, P, lambda kd: x2_first[kd])
                elif ch == 1:
                    proj_bf16(pk, 2 * DK, P, lambda kd: x2b1[:, kd, :])
                else:
                    proj_dr(pk, 2 * DK, P,
                            x2t8s[(ch - 2) // 2][:, (ch - 2) % 2])
                bias_relu(kT_sb[:, ch * CH:(ch + 1) * CH], pk, b_sb[:, 1:2])
                pv = pps.tile([DK, CH], F32, tag="pps")
                if ch == 0:
                    proj_bf16(pv, 4 * DK, DK, lambda kd: x2_first[kd])
                elif ch == 1:
                    proj_bf16(pv, 4 * DK, DK, lambda kd: x2b1[:, kd, :])
                else:
                    proj_dr(pv, 4 * DK, DK,
                            x2t8s[(ch - 2) // 2][:, (ch - 2) % 2])
                bias_relu(vT_sb[:, ch * CH:(ch + 1) * CH], pv, b_sb[0:DK, 2:3])

            def transpose_v(st):
                pt = pps.tile([P, DK], BF16, tag="pps")
                nc.tensor.transpose(
                    pt, in_=vT_sb[:, st * P:(st + 1) * P],
                    identity=identv[:DK, :DK],
                )
                nc.vector.tensor_copy(v_sb[:, st, 0:DK], pt)

            def finalize_job(j, oT_ps):
                oT = ostage.tile([DK + 1, CH], F32, tag="oT")
                for h in range(2):
                    nc.scalar.activation(
                        out=oT[:, h * 256:(h + 1) * 256],
                        in_=oT_ps[:, h * 256:(h + 1) * 256],
                        func=mybir.ActivationFunctionType.Copy,
                        bias=0.0, scale=1.0,
                    )
                ot4 = ostage.tile([P, CH // P, DK], F32, tag="ot4")
                for blk in range(CH // P):
                    po = pps.tile([P, DK + 1], F32, tag="pps")
                    nc.tensor.transpose(
                        po,
                        in_=oT[:, blk * P:(blk + 1) * P],
                        identity=ident[:DK + 1, :DK + 1],
                    )
                    rec = ostage.tile([P, 1], F32, tag="rec")
                    nc.vector.reciprocal(rec, po[:, DK:DK + 1])
                    nc.vector.tensor_scalar_mul(ot4[:, blk, :], po[:, 0:DK],
                                                rec)
                nc.sync.dma_start(
                    out=out[j].rearrange("(blk p) d -> p blk d", p=P),
                    in_=ot4,
                )

            def attention_job(j, new_tiles=(), finalize_prev=None):
                oT_ps = outps.tile([DK + 1, CH], F32, tag="outT")
                qslc = qT_sb[:, j * CH:(j + 1) * CH]
                npair = E[j] // 2
                DEPTH = 2
                pending = []
                for pi in range(npair + DEPTH):
                    for st in new_tiles[2 * pi:2 * pi + 2]:
                        transpose_v(st)
                    if pi == 1 and finalize_prev is not None:
                        finalize_prev()
                    if pi < npair:
                        sc = sps.tile([P, 1024], F32, tag="sc")
                        at = attn.tile([P, 1024], BF16, tag="attnT")
                        for half in range(2):
                            t = 2 * pi + half
                            lo = half * DK
                            nc.tensor.matmul(
                                sc[:, half * CH:(half + 1) * CH],
                                kT_sb[lo:lo + DK, t * P:(t + 1) * P],
                                qslc[lo:lo + DK, :],
                                start=True,
                                stop=True,
                            )
                        nc.scalar.activation(
                            out=at, in_=sc, func=Exp, scale=ESCALE
                        )
                        halves = []
                        for half in range(2):
                            t = 2 * pi + half
                            aslc = at[:, half * CH:(half + 1) * CH]
                            if causal and t >= E[j] - 8:
                                m = t - (E[j] - 8)
                                nc.vector.tensor_tensor(
                                    aslc, aslc, masks_sb[:, m, :],
                                    mybir.AluOpType.mult,
                                )
                            halves.append((t, aslc))
                        pending.append(halves)
                    if pi >= DEPTH:
                        for t, aslc in pending.pop(0):
                            nc.tensor.matmul(
                                oT_ps,
                                v_sb[:, t, 0:DK + 1],
                                aslc,
                                start=(t == 0),
                                stop=(t == E[j] - 1),
                                skip_group_check=True,
                            )
                return lambda: finalize_job(j, oT_ps)

            # ---------------- interleaved emission: group j feeds job j
            fin = None
            for j in range(NJ):
                proj_q_chunk(j)
                lo, hi = 2 * j, 2 * j + 2
                if not causal:
                    lo, hi = (0, S // CH) if j == 0 else (0, 0)
                new_tiles = []
                for ch in range(lo, hi):
                    proj_kv_chunk(ch)
                    new_tiles.extend(
                        ch * (CH // P) + blk for blk in range(CH // P)
                    )
                if not causal and j == 0:
                    for st in new_tiles:
                        transpose_v(st)
                    new_tiles = []
                fin = attention_job(j, new_tiles, finalize_prev=fin)
            fin()

    _split_sync_waits(nc)
    return nc


_PROGRAMS = {}


def _program(causal: bool):
    if causal not in _PROGRAMS:
        _PROGRAMS[causal] = _build_program(causal)
    return _PROGRAMS[causal]


def _host_masks(parity: int) -> np.ndarray:
    """masks[m] multiplies the exp'd [sk=128, sq=512] tile of the job whose
    diagonal band covers key tiles [E-8, E); m = position in that band."""
    sk = np.arange(P)[:, None]
    sq = np.arange(CH)[None, :]
    m = np.zeros((8, P, CH), np.float32)
    for i in range(8):
        if parity == 1:
            if i < 4:
                m[i] = 1.0
            else:
                r = i - 4
                m[i] = (sq >= r * P + sk).astype(np.float32)
        else:
            if i < 4:
                m[i] = (sq >= i * P + sk).astype(np.float32)
            else:
                m[i] = 0.0
    return m


def _chunked(xt_rows: np.ndarray, np_x) -> np.ndarray:
    """[rows, D] -> [nch, 128, KD*CH] where [ch, p, kd*CH+s] =
    x[ch*CH+s, kd*128+p]."""
    nch = xt_rows.shape[0] // CH
    a = xt_rows.reshape(nch, CH, KD, P).transpose(0, 3, 2, 1)
    return np.ascontiguousarray(a.reshape(nch, P, KD * CH).astype(np_x))


def kernel(x1, x2, Wq, bq, Wk, bk, Wv, bv, apply_mask):
    x1 = np.asarray(x1, dtype=np.float32)
    x2 = np.asarray(x2, dtype=np.float32)
    Wq_f = np.asarray(Wq, np.float32)
    Wk_f = np.asarray(Wk, np.float32)
    Wv_f = np.asarray(Wv, np.float32)
    Wcat = WSCALE * np.concatenate([Wq_f, Wq_f, Wk_f, Wk_f, Wv_f], axis=1)
    wall_f = np.ascontiguousarray(
        Wcat.reshape(KD, P, WM).transpose(1, 0, 2).reshape(P, KD * WM)
    )
    ball_h = np.zeros((P, 3), np.float32)
    ball_h[:, 0] = WSCALE * np.concatenate([bq, bq])
    ball_h[:, 1] = WSCALE * np.concatenate([bk, bk])
    ball_h[0:DK, 2] = WSCALE * np.asarray(bv, np.float32)
    causal = bool(int(np.asarray(apply_mask)))

    nc = _program(causal)

    x2cb_h = [_chunked(x2[b][:2 * CH], NPB) for b in range(B)]
    x2c8_h = [_chunked(x2[b][2 * CH:], NP8) for b in range(B)]
    ones_h = np.full((P, NSK), WSCALE, NPB)
    masks_f = [_host_masks(p).astype(NPB) for p in range(2)]

    in_maps = []
    for core in range(N_CORES):
        b, p = core // 2, core % 2
        xb = x1[b]
        rows = np.concatenate(
            [xb[(2 * j + p) * CH:(2 * j + p + 1) * CH] for j in range(NJ)],
            axis=0,
        )
        in_maps.append({
            "x1b": _chunked(rows[:CH], NPB)[0],
            "x1e": _chunked(rows[CH:], NP8),
            "x2b": x2cb_h[b],
            "x2e": x2c8_h[b],
            "wallb": wall_f.astype(NPB),
            "wall8": wall_f.astype(NP8),
            "ball": ball_h,
            "masks": masks_f[p],
            "ones": ones_h,
        })

    res = run_bass_kernel_spmd(
        nc, in_maps, core_ids=list(range(N_CORES)), trace=_CFG["trace"]
    )
    kernel.last_result = res

    outp = np.empty((B, S, DK), np.float32)
    for core in range(N_CORES):
        b, p = core // 2, core % 2
        o = res.results[core]["out"].reshape(SQ, DK)
        for j in range(NJ):
            outp[b, (2 * j + p) * CH:(2 * j + p + 1) * CH] = \
                o[j * CH:(j + 1) * CH]
    return outp
